# revision 25
# baseline (speedup 1.0000x reference)
"""ALiBi attention (B=2, S=2048, C=1024, H=16) on 8 trn2 NeuronCores.

Sharding: head-parallel. Core c owns heads (c, c+8) for both batches:
  - in_proj computed per-core only for its 6 head-slices (q,k,v x 2 heads),
    directly in transposed [channel, token] layout (x is host-transposed and
    host-cast to bf16; all matmul operands are bf16/fp16 so the PE uses fast
    weight loads).
  - scores are computed transposed (S^T[j,i] = k_j . q_i) so softmax j-sums
    come from a ones-column augmented onto v, and the probability matrix is
    never transposed.
  - ALiBi bias min(slope*(i-j), 8) becomes a multiply with a host-precomputed
    exp(bias-8) fp16 table; saturated tiles skip the multiply (the +8 cancels
    against the exp's -8 range shift) and far-future tiles are skipped
    entirely. Classifications depend only on the head SLOT, keeping the SPMD
    program valid on every core.
  - elementwise work is spread across engines: exp on Scalar, most bias
    multiplies on Vector (some on GpSimd), qkv/py PSUM evacuation on
    GpSimd/Vector, so no single engine serializes the pipeline.
  - out_proj is row-parallel: each core emits a partial y in fp16; the host
    sums the 8 partials and adds out_proj_bias (the "all-reduce").
"""
import functools
import math
import sys

sys.path.insert(0, "/opt/trn_rl_repo")

import numpy as np

B, S, C, H, D = 2, 2048, 1024, 16, 64
TOK = B * S
NCORE = 8
MAX_BIAS = 8.0
BTW = 2 * S - 128       # shifted bias-table width (full, for slot-1 heads)
BT0_OFF = 384           # slot-0 table column offset (unfolded tiles only)
BT0_W = 2816            # slot-0 table width
SCALE = float(D) ** -0.5
HALF_SKIP = 704         # skip (j, iq) half if j0 - iq_abs0 >= this (slot 0)
FOLD_I_MINUS_J = 255    # inject-free tile if i0 - j0 >= this (slot 0 only)


def _slopes() -> np.ndarray:
    start = 2.0 ** (-(2.0 ** (-(math.log2(H) - 3))))
    return np.array([start * start**i for i in range(H)], dtype=np.float32)


@functools.lru_cache(maxsize=1)
def _program():
    import concourse.mybir as mybir
    import concourse.tile as tile
    from concourse import bacc
    from concourse.masks import make_identity

    F32 = mybir.dt.float32
    BF16 = mybir.dt.bfloat16
    F16 = mybir.dt.float16
    Exp = mybir.ActivationFunctionType.Exp
    Ident = mybir.ActivationFunctionType.Identity
    MUL = mybir.AluOpType.mult

    nc = bacc.Bacc("TRN2", target_bir_lowering=False, debug=False)

    xt = nc.dram_tensor("xt", [C, TOK], BF16, kind="ExternalInput").ap()
    wqkvt = nc.dram_tensor("wqkvt", [C, 384], BF16, kind="ExternalInput").ap()
    bqkv = nc.dram_tensor("bqkv", [128, 3], F32, kind="ExternalInput").ap()
    bt = nc.dram_tensor("bt", [2, 128, BTW], F16, kind="ExternalInput").ap()
    wot = nc.dram_tensor("wot", [128, C], BF16, kind="ExternalInput").ap()
    y = nc.dram_tensor("y", [TOK, C], F16, kind="ExternalOutput").ap()

    with tile.TileContext(nc) as tc:
        with tc.tile_pool(name="const", bufs=1) as cpool, \
             tc.tile_pool(name="wpool", bufs=1) as wpool, \
             tc.tile_pool(name="qkvp", bufs=1) as qkvp, \
             tc.tile_pool(name="xin", bufs=4) as xpool, \
             tc.tile_pool(name="probs", bufs=2) as ppool, \
             tc.tile_pool(name="work", bufs=2) as wk, \
             tc.tile_pool(name="ps", bufs=2, space="PSUM") as ps:

            wq_sb = wpool.tile([128, 8, 384], BF16, name="wq_sb")
            wqkvt_r = wqkvt.rearrange("(co p) n -> p co n", p=128)
            for cb in range(8):
                nc.sync.dma_start(wq_sb[:, cb:cb + 1, :],
                                  wqkvt_r[:, cb:cb + 1, :])
            bq_sb = wpool.tile([128, 3], F32, name="bq_sb")
            nc.sync.dma_start(bq_sb[:], bqkv)
            ident = cpool.tile([128, 128], F32, name="ident")
            make_identity(nc, ident[:])
            identb = cpool.tile([128, 128], BF16, name="identb")
            nc.vector.tensor_copy(identb[:], ident[:])
            btab1 = wpool.tile([128, BTW], F16, name="btab1")
            btab0 = wpool.tile([128, BT0_W], F16, name="btab0")
            wo_sb = wpool.tile([128, C], BF16, name="wo_sb")

            def load_tables():
                nc.sync.dma_start(btab1[:],
                                  bt.rearrange("h p c -> p h c")[:, 1, :])
                nc.sync.dma_start(
                    btab0[:],
                    bt.rearrange("h p c -> p h c")[:, 0,
                                                   BT0_OFF:BT0_OFF + BT0_W])
                nc.sync.dma_start(wo_sb[:], wot)

            qkvT = qkvp.tile([128, 3, TOK], BF16, name="qkvT")
            kpadB = qkvp.tile([128, TOK], BF16, name="kpadB")
            nc.scalar.memzero(qkvT[64:128, 1, :])
            nc.scalar.memzero(kpadB[0:64, :])
            v_nat = qkvp.tile([128, 32, 2, 65], F16, name="v_nat")
            nc.vector.memset(v_nat[:, :, :, 64:65], 1.0)
            oT = qkvp.tile([128, TOK], BF16, name="oT")

            xt_r = xt.rearrange("(co p) t -> p co t", p=128)
            y_r = y.rearrange("(tb p) c -> tb p c", p=128)

            # in_proj / v_transpose / out_proj are emitted as "chunk" lists:
            # either run back-to-back (opening phase) or interleaved one
            # chunk per j-tile inside the attention loops, filling the
            # engine gaps left by the per-tile score->exp->mult->PV chain.
            xt_hold = {}

            def in_proj_chunks(bb, evac_dve):
                def dma_chunk(tb):
                    def go():
                        xtile = xpool.tile([128, 8, 512], BF16,
                                           name=f"xt{tb}", tag="xtile")
                        ts = slice(tb * 512, (tb + 1) * 512)
                        nc.sync.dma_start(xtile[:, 0:4, :],
                                          xt_r[:, 0:4, ts])
                        nc.sync.dma_start(xtile[:, 4:8, :],
                                          xt_r[:, 4:8, ts])
                        xt_hold[tb] = xtile
                    return go

                def mm_chunk(tb, chb, tag):
                    def go():
                        pin = ps.tile([128, 512], F32, name=f"pin{tb}_{chb}",
                                      tag=tag)
                        for cb in range(8):
                            nc.tensor.matmul(
                                pin[:],
                                wq_sb[:, cb, chb * 128:(chb + 1) * 128],
                                xt_hold[tb][:, cb, :],
                                start=(cb == 0), stop=(cb == 7))
                        ts = slice(tb * 512, (tb + 1) * 512)
                        if evac_dve:
                            with nc.allow_low_precision(reason="bf16 qkv"):
                                if chb == 1:
                                    nc.vector.tensor_scalar_add(
                                        qkvT[0:64, 1, ts], pin[0:64],
                                        bq_sb[0:64, 1:2])
                                    nc.vector.tensor_scalar_add(
                                        kpadB[64:128, ts], pin[64:128],
                                        bq_sb[64:128, 1:2])
                                else:
                                    nc.vector.tensor_scalar_add(
                                        qkvT[:, chb, ts], pin[:],
                                        bq_sb[:, chb:chb + 1])
                        elif chb == 1:
                            nc.scalar.activation(
                                qkvT[0:64, 1, ts], pin[0:64], Ident,
                                bias=bq_sb[0:64, 1:2], scale=1.0)
                            nc.scalar.activation(
                                kpadB[64:128, ts], pin[64:128], Ident,
                                bias=bq_sb[64:128, 1:2], scale=1.0)
                        else:
                            nc.scalar.activation(
                                qkvT[:, chb, ts], pin[:], Ident,
                                bias=bq_sb[:, chb:chb + 1], scale=1.0)
                    return go

                chunks = [dma_chunk(tb) for tb in range(4 * bb, 4 * bb + 4)]
                for ti, tb in enumerate(range(4 * bb, 4 * bb + 4)):
                    for chb in range(3):
                        # opening phase alternates two PSUM rings; the
                        # interleaved phase keeps "sc" free for score tiles
                        tag = "py" if evac_dve or (ti * 3 + chb) % 2 else "sc"
                        chunks.append(mm_chunk(tb, chb, tag))
                return chunks

            def vt_chunks(bb, tag_alt):
                def one(t32, tag):
                    def go():
                        pv = ps.tile([128, 128], BF16, name=f"pv{t32}",
                                     tag=tag)
                        nc.tensor.transpose(
                            pv[:],
                            qkvT[:, 2, t32 * 128:(t32 + 1) * 128],
                            identb[:])
                        with nc.allow_low_precision(reason="fp16 v"):
                            for hh in range(2):
                                nc.vector.tensor_copy(
                                    v_nat[:, t32, hh, 0:64],
                                    pv[:, hh * 64:hh * 64 + 64])
                    return go
                return [one(t32, "py" if not tag_alt or t32 % 2 else "sc")
                        for t32 in range(16 * bb, 16 * bb + 16)]

            def attn_iter(b, ih, hh, side=()):
                side = list(side)
                hb = hh * 64
                i0 = ih * 1024
                it = f"{b}{ih}{hh}"

                def alive(j):
                    if hh == 1:
                        return (0, 1)
                    return tuple(iq for iq in (0, 1)
                                 if j * 128 - (i0 + iq * 512) < HALF_SKIP)

                js = [j for j in range(16) if alive(j)]
                last_j = {iq: max(j for j in js if iq in alive(j))
                          for iq in (0, 1)}
                pacc = ps.tile([65, 1024], F32, name=f"pa{it}", tag="acc",
                               bufs=1)
                started = [False, False]
                pend = []  # PV queue, depth 4 hides the exp->EB-mult chain

                def flush_pv():
                    pvb, pvj, halves = pend.pop(0)
                    for iq in halves:
                        nc.tensor.matmul(pacc[:, iq * 512:(iq + 1) * 512],
                                         v_nat[:, b * 16 + pvj, hh, :],
                                         pvb[:, iq * 512:(iq + 1) * 512],
                                         start=not started[iq],
                                         stop=pvj == last_j[iq])
                        started[iq] = True

                for j in js:
                    j0 = j * 128
                    halves = alive(j)
                    fold = hh == 0 and i0 - j0 >= FOLD_I_MINUS_J
                    pS = ps.tile([128, 1024], F32, name=f"pS{it}_{j}", tag="sc")
                    if hh == 0:
                        kT = qkvT[:, 1, b * 2048 + j0: b * 2048 + j0 + 128]
                    else:
                        kT = kpadB[:, b * 2048 + j0: b * 2048 + j0 + 128]
                    for iq in halves:
                        ii = i0 + iq * 512
                        sl = pS[:, iq * 512:(iq + 1) * 512]
                        qT = qkvT[:, 0, b * 2048 + ii: b * 2048 + ii + 512]
                        nc.tensor.matmul(sl, kT, qT, start=True, stop=True)
                    # probs = exp(s) * exp(bias-8): same value range as the
                    # additive exp(s+bias-8); the fp16 table multiply runs on
                    # the DVE instead of PE identity-injects.
                    pb = ppool.tile([128, 1024], F16, name=f"pb{it}_{j}",
                                    tag="pb", bufs=7)
                    psl = (slice(0, 1024) if halves == (0, 1)
                           else slice(halves[0] * 512, halves[0] * 512 + 512))
                    nc.scalar.activation(pb[:, psl], pS[:, psl], Exp,
                                         bias=0.0, scale=1.0)
                    if not fold:
                        c0 = i0 - j0 + (S - 128)
                        w0, w1 = psl.start, psl.stop
                        if hh == 0:
                            eb = btab0[:, c0 - BT0_OFF + w0:c0 - BT0_OFF + w1]
                        else:
                            eb = btab1[:, c0 + w0:c0 + w1]
                        with nc.allow_low_precision(reason="fp16 probs"):
                            nc.vector.tensor_tensor(pb[:, psl], pb[:, psl],
                                                    eb, MUL)
                    if len(pend) == 5:
                        flush_pv()
                    pend.append((pb, j, halves))
                    if side:
                        side.pop(0)()
                while pend:
                    flush_pv()
                # normalization: oT = pacc[0:64] * (1/rowsum). pacc is freed
                # by the two copies below; the reciprocal round-trip and the
                # in-place multiply run off the critical path (reciprocal in
                # [8,128] layout, row<->col reshapes on DMA, broadcast on the
                # idle GpSimd engine).
                sumr = wk.tile([1, 1024], F32, name=f"sr{it}", tag="sumr",
                               bufs=1)
                nc.vector.tensor_copy(sumr[:], pacc[64:65, :])
                osl = oT[hb:hb + 64, b * 2048 + i0: b * 2048 + i0 + 1024]
                with nc.allow_low_precision(reason="bf16 out"):
                    nc.vector.tensor_copy(osl, pacc[0:64, :])
                sumc = wk.tile([8, 128], F32, name=f"sc{it}", tag="sumc")
                nc.sync.dma_start(sumc[:],
                                  sumr[:].rearrange("o (p a) -> o p a", a=128))
                inv8 = wk.tile([8, 128], F32, name=f"i8{it}", tag="inv8")
                nc.vector.reciprocal(inv8[:], sumc[:])
                invr = wk.tile([1, 1024], F32, name=f"iv{it}", tag="invr",
                               bufs=1)
                nc.sync.dma_start(invr[:].rearrange("o (p a) -> o p a", a=128),
                                  inv8[:])
                invbc = wk.tile([128, 1024], F32, name=f"ib{it}", tag="invbc",
                                bufs=1)
                nc.gpsimd.partition_broadcast(invbc[:], invr[:], channels=128)
                with nc.allow_low_precision(reason="bf16 out"):
                    nc.vector.tensor_tensor(osl, osl, invbc[hb:hb + 64, :],
                                            MUL)
                while side:
                    side.pop(0)()

            def out_proj_chunks(b, ih):
                def one(tloc):
                    def go():
                        tb = b * 16 + ih * 8 + tloc
                        ytile = wk.tile([128, 1024], F16, name=f"yt{tb}",
                                        tag="ytile", bufs=3)
                        for cq in range(2):
                            py_ = ps.tile([128, 512], F32,
                                          name=f"py{tb}_{cq}", tag="py")
                            nc.tensor.matmul(
                                py_[:],
                                oT[:, tb * 128:(tb + 1) * 128],
                                wo_sb[:, cq * 512:(cq + 1) * 512],
                                start=True, stop=True)
                            # split the evacuation: DVE is the binding
                            # engine in the out_proj windows, Scalar has
                            # slack there
                            if cq == 0:
                                with nc.allow_low_precision(reason="fp16 y"):
                                    nc.vector.tensor_copy(
                                        ytile[:, 0:512], py_[:])
                            else:
                                nc.scalar.activation(
                                    ytile[:, 512:1024], py_[:], Ident,
                                    bias=0.0, scale=1.0)
                            nc.sync.dma_start(
                                y_r[tb][:, cq * 512:(cq + 1) * 512],
                                ytile[:, cq * 512:(cq + 1) * 512])
                    return go
                return [one(tloc) for tloc in range(8)]

            ip0 = in_proj_chunks(0, evac_dve=False)
            for c in ip0:
                c()
            load_tables()
            for c in vt_chunks(0, tag_alt=True):
                c()
            ip1 = in_proj_chunks(1, evac_dve=True)
            attn_iter(0, 0, 0, side=ip1[:8])
            attn_iter(0, 0, 1)
            attn_iter(0, 1, 0, side=ip1[8:])
            vt1 = vt_chunks(1, tag_alt=False)
            attn_iter(0, 1, 1, side=vt1)
            attn_iter(1, 0, 0)
            attn_iter(1, 0, 1, side=out_proj_chunks(0, 0))
            attn_iter(1, 1, 0, side=out_proj_chunks(0, 1))
            attn_iter(1, 1, 1, side=out_proj_chunks(1, 0))
            for c in out_proj_chunks(1, 1):
                c()

    nc.compile()
    return nc


def _make_inmaps(x, in_proj_weight, in_proj_bias, out_proj_weight):
    import ml_dtypes
    bf16 = ml_dtypes.bfloat16

    slopes = _slopes()
    xT = np.ascontiguousarray(
        x.reshape(TOK, C).T.astype(bf16))  # [C, TOK]

    in_maps = []
    p = np.arange(128, dtype=np.float64)[:, None]
    cc = np.arange(BTW, dtype=np.float64)[None, :]
    for c in range(NCORE):
        heads = (c, c + 8)
        rows = []
        for sec in range(3):  # q, k, v
            for h in heads:
                rows.extend(range(sec * C + h * D, sec * C + (h + 1) * D))
        rows = np.array(rows)
        wq = in_proj_weight[rows, :].astype(np.float32).copy()
        bq = in_proj_bias[rows].astype(np.float32).copy()
        wq[:128] *= SCALE  # fold q scaling
        bq[:128] *= SCALE
        wqkvt = np.ascontiguousarray(wq.T.astype(bf16))  # [C, 384]
        bqkv = np.ascontiguousarray(bq.reshape(3, 128).T)  # [128, 3]

        btarr = np.empty((2, 128, BTW), dtype=np.float16)
        for hh, h in enumerate(heads):
            bias = np.minimum(float(slopes[h]) * (cc - (S - 128) - p),
                              float(MAX_BIAS))
            btarr[hh] = np.exp(bias - float(MAX_BIAS)).astype(np.float16)

        ocols = np.array(
            [heads[0] * D + d for d in range(D)]
            + [heads[1] * D + d for d in range(D)]
        )
        wotr = np.ascontiguousarray(
            out_proj_weight[:, ocols].T.astype(bf16))  # [128, C]

        in_maps.append({
            "xt": xT,
            "wqkvt": wqkvt,
            "bqkv": bqkv,
            "bt": btarr,
            "wot": wotr,
        })
    return in_maps


def run(inputs: dict, trace: bool = False):
    from concourse.bass_utils import run_bass_kernel_spmd

    nc = _program()
    in_maps = _make_inmaps(
        np.asarray(inputs["x"]),
        np.asarray(inputs["in_proj_weight"]),
        np.asarray(inputs["in_proj_bias"]),
        np.asarray(inputs["out_proj_weight"]),
    )
    res = run_bass_kernel_spmd(nc, in_maps, list(range(NCORE)), trace=trace)
    acc = np.zeros((TOK, C), dtype=np.float64)
    for r in res.results:
        acc += r["y"].astype(np.float64)
    acc += np.asarray(inputs["out_proj_bias"]).astype(np.float64)[None, :]
    out = acc.astype(np.float32).reshape(B, S, C)
    return out, res


def kernel(**inputs) -> np.ndarray:
    return run(inputs, trace=False)[0]


# revision 26
# speedup vs baseline: 1.0113x; 1.0113x over previous
"""ALiBi attention (B=2, S=2048, C=1024, H=16) on 8 trn2 NeuronCores.

Sharding: head-parallel. Core c owns heads (c, c+8) for both batches:
  - in_proj computed per-core only for its 6 head-slices (q,k,v x 2 heads),
    directly in transposed [channel, token] layout (x is host-transposed and
    host-cast to bf16; all matmul operands are bf16/fp16 so the PE uses fast
    weight loads).
  - scores are computed transposed (S^T[j,i] = k_j . q_i) so softmax j-sums
    come from a ones-column augmented onto v, and the probability matrix is
    never transposed.
  - ALiBi bias min(slope*(i-j), 8) becomes a multiply with a host-precomputed
    exp(bias-8) fp16 table; saturated tiles skip the multiply (the +8 cancels
    against the exp's -8 range shift) and far-future tiles are skipped
    entirely. Classifications depend only on the head SLOT, keeping the SPMD
    program valid on every core.
  - elementwise work is spread across engines: exp on Scalar, most bias
    multiplies on Vector (some on GpSimd), qkv/py PSUM evacuation on
    GpSimd/Vector, so no single engine serializes the pipeline.
  - out_proj is row-parallel: each core emits a partial y in fp16; the host
    sums the 8 partials and adds out_proj_bias (the "all-reduce").
"""
import functools
import math
import sys

sys.path.insert(0, "/opt/trn_rl_repo")

import numpy as np

B, S, C, H, D = 2, 2048, 1024, 16, 64
TOK = B * S
NCORE = 8
MAX_BIAS = 8.0
BTW = 2 * S - 128       # shifted bias-table width (full, for slot-1 heads)
BT0_OFF = 384           # slot-0 table column offset (unfolded tiles only)
BT0_W = 2816            # slot-0 table width
SCALE = float(D) ** -0.5
HALF_SKIP = 704         # skip (j, iq) half if j0 - iq_abs0 >= this (slot 0)
FOLD_I_MINUS_J = 255    # inject-free tile if i0 - j0 >= this (slot 0 only)


def _slopes() -> np.ndarray:
    start = 2.0 ** (-(2.0 ** (-(math.log2(H) - 3))))
    return np.array([start * start**i for i in range(H)], dtype=np.float32)


@functools.lru_cache(maxsize=1)
def _program():
    import concourse.mybir as mybir
    import concourse.tile as tile
    from concourse import bacc
    from concourse.masks import make_identity

    F32 = mybir.dt.float32
    BF16 = mybir.dt.bfloat16
    F16 = mybir.dt.float16
    Exp = mybir.ActivationFunctionType.Exp
    Ident = mybir.ActivationFunctionType.Identity
    MUL = mybir.AluOpType.mult

    nc = bacc.Bacc("TRN2", target_bir_lowering=False, debug=False)

    xt = nc.dram_tensor("xt", [C, TOK], BF16, kind="ExternalInput").ap()
    wqkvt = nc.dram_tensor("wqkvt", [C, 384], BF16, kind="ExternalInput").ap()
    bqkv = nc.dram_tensor("bqkv", [128, 3], F32, kind="ExternalInput").ap()
    bt = nc.dram_tensor("bt", [2, 128, BTW], F16, kind="ExternalInput").ap()
    wot = nc.dram_tensor("wot", [128, C], BF16, kind="ExternalInput").ap()
    y = nc.dram_tensor("y", [TOK, C], F16, kind="ExternalOutput").ap()

    with tile.TileContext(nc) as tc:
        with tc.tile_pool(name="const", bufs=1) as cpool, \
             tc.tile_pool(name="wpool", bufs=1) as wpool, \
             tc.tile_pool(name="qkvp", bufs=1) as qkvp, \
             tc.tile_pool(name="xin", bufs=4) as xpool, \
             tc.tile_pool(name="probs", bufs=2) as ppool, \
             tc.tile_pool(name="work", bufs=2) as wk, \
             tc.tile_pool(name="ps", bufs=2, space="PSUM") as ps:

            wq_sb = wpool.tile([128, 8, 384], BF16, name="wq_sb")
            wqkvt_r = wqkvt.rearrange("(co p) n -> p co n", p=128)
            nc.sync.dma_start(wq_sb[:, 0:1, :], wqkvt_r[:, 0:1, :])
            bq_sb = wpool.tile([128, 3], F32, name="bq_sb")
            ident = cpool.tile([128, 128], F32, name="ident")
            make_identity(nc, ident[:])
            identb = cpool.tile([128, 128], BF16, name="identb")
            nc.vector.tensor_copy(identb[:], ident[:])
            btab1 = wpool.tile([128, BTW], F16, name="btab1")
            btab0 = wpool.tile([128, BT0_W], F16, name="btab0")
            wo_sb = wpool.tile([128, C], BF16, name="wo_sb")

            def load_tables():
                nc.sync.dma_start(btab1[:],
                                  bt.rearrange("h p c -> p h c")[:, 1, :])
                nc.sync.dma_start(
                    btab0[:],
                    bt.rearrange("h p c -> p h c")[:, 0,
                                                   BT0_OFF:BT0_OFF + BT0_W])
                nc.sync.dma_start(wo_sb[:], wot)

            qkvT = qkvp.tile([128, 3, TOK], BF16, name="qkvT")
            kpadB = qkvp.tile([128, TOK], BF16, name="kpadB")
            nc.scalar.memzero(qkvT[64:128, 1, :])
            nc.scalar.memzero(kpadB[0:64, :])
            v_nat = qkvp.tile([128, 32, 2, 65], F16, name="v_nat")
            nc.vector.memset(v_nat[:, :, :, 64:65], 1.0)
            oT = qkvp.tile([128, TOK], BF16, name="oT")

            xt_r = xt.rearrange("(co p) t -> p co t", p=128)
            y_r = y.rearrange("(tb p) c -> tb p c", p=128)

            # in_proj / v_transpose / out_proj are emitted as "chunk" lists:
            # either run back-to-back (opening phase) or interleaved one
            # chunk per j-tile inside the attention loops, filling the
            # engine gaps left by the per-tile score->exp->mult->PV chain.
            xt_hold = {}

            def in_proj_chunks(bb, evac_dve):
                def dma_chunk(tb):
                    def go():
                        xtile = xpool.tile([128, 8, 512], BF16,
                                           name=f"xt{tb}", tag="xtile")
                        ts = slice(tb * 512, (tb + 1) * 512)
                        nc.sync.dma_start(xtile[:, 0:4, :],
                                          xt_r[:, 0:4, ts])
                        nc.sync.dma_start(xtile[:, 4:8, :],
                                          xt_r[:, 4:8, ts])
                        xt_hold[tb] = xtile
                    return go

                def mm_chunk(tb, chb, tag):
                    def go():
                        pin = ps.tile([128, 512], F32, name=f"pin{tb}_{chb}",
                                      tag=tag)
                        for cb in range(8):
                            nc.tensor.matmul(
                                pin[:],
                                wq_sb[:, cb, chb * 128:(chb + 1) * 128],
                                xt_hold[tb][:, cb, :],
                                start=(cb == 0), stop=(cb == 7))
                        ts = slice(tb * 512, (tb + 1) * 512)
                        if evac_dve:
                            with nc.allow_low_precision(reason="bf16 qkv"):
                                if chb == 1:
                                    nc.vector.tensor_scalar_add(
                                        qkvT[0:64, 1, ts], pin[0:64],
                                        bq_sb[0:64, 1:2])
                                    nc.vector.tensor_scalar_add(
                                        kpadB[64:128, ts], pin[64:128],
                                        bq_sb[64:128, 1:2])
                                else:
                                    nc.vector.tensor_scalar_add(
                                        qkvT[:, chb, ts], pin[:],
                                        bq_sb[:, chb:chb + 1])
                        elif chb == 1:
                            nc.scalar.activation(
                                qkvT[0:64, 1, ts], pin[0:64], Ident,
                                bias=bq_sb[0:64, 1:2], scale=1.0)
                            nc.scalar.activation(
                                kpadB[64:128, ts], pin[64:128], Ident,
                                bias=bq_sb[64:128, 1:2], scale=1.0)
                        else:
                            nc.scalar.activation(
                                qkvT[:, chb, ts], pin[:], Ident,
                                bias=bq_sb[:, chb:chb + 1], scale=1.0)
                    return go

                chunks = [dma_chunk(tb) for tb in range(4 * bb, 4 * bb + 4)]
                for ti, tb in enumerate(range(4 * bb, 4 * bb + 4)):
                    for chb in range(3):
                        # opening phase alternates two PSUM rings; the
                        # interleaved phase keeps "sc" free for score tiles
                        tag = "py" if evac_dve or (ti * 3 + chb) % 2 else "sc"
                        chunks.append(mm_chunk(tb, chb, tag))
                return chunks

            def vt_chunks(bb, tag_alt):
                def one(t32, tag):
                    def go():
                        pv = ps.tile([128, 128], BF16, name=f"pv{t32}",
                                     tag=tag)
                        nc.tensor.transpose(
                            pv[:],
                            qkvT[:, 2, t32 * 128:(t32 + 1) * 128],
                            identb[:])
                        with nc.allow_low_precision(reason="fp16 v"):
                            for hh in range(2):
                                nc.vector.tensor_copy(
                                    v_nat[:, t32, hh, 0:64],
                                    pv[:, hh * 64:hh * 64 + 64])
                    return go
                return [one(t32, "py" if not tag_alt or t32 % 2 else "sc")
                        for t32 in range(16 * bb, 16 * bb + 16)]

            def attn_iter(b, ih, hh, side=()):
                side = list(side)
                hb = hh * 64
                i0 = ih * 1024
                it = f"{b}{ih}{hh}"

                def alive(j):
                    if hh == 1:
                        return (0, 1)
                    return tuple(iq for iq in (0, 1)
                                 if j * 128 - (i0 + iq * 512) < HALF_SKIP)

                js = [j for j in range(16) if alive(j)]
                last_j = {iq: max(j for j in js if iq in alive(j))
                          for iq in (0, 1)}
                pacc = ps.tile([65, 1024], F32, name=f"pa{it}", tag="acc",
                               bufs=1)
                started = [False, False]
                pend = []  # PV queue, depth 4 hides the exp->EB-mult chain

                def flush_pv():
                    pvb, pvj, halves = pend.pop(0)
                    for iq in halves:
                        nc.tensor.matmul(pacc[:, iq * 512:(iq + 1) * 512],
                                         v_nat[:, b * 16 + pvj, hh, :],
                                         pvb[:, iq * 512:(iq + 1) * 512],
                                         start=not started[iq],
                                         stop=pvj == last_j[iq])
                        started[iq] = True

                for j in js:
                    j0 = j * 128
                    halves = alive(j)
                    fold = hh == 0 and i0 - j0 >= FOLD_I_MINUS_J
                    pS = ps.tile([128, 1024], F32, name=f"pS{it}_{j}", tag="sc")
                    if hh == 0:
                        kT = qkvT[:, 1, b * 2048 + j0: b * 2048 + j0 + 128]
                    else:
                        kT = kpadB[:, b * 2048 + j0: b * 2048 + j0 + 128]
                    for iq in halves:
                        ii = i0 + iq * 512
                        sl = pS[:, iq * 512:(iq + 1) * 512]
                        qT = qkvT[:, 0, b * 2048 + ii: b * 2048 + ii + 512]
                        nc.tensor.matmul(sl, kT, qT, start=True, stop=True)
                    # probs = exp(s) * exp(bias-8): same value range as the
                    # additive exp(s+bias-8); the fp16 table multiply runs on
                    # the DVE instead of PE identity-injects.
                    pb = ppool.tile([128, 1024], F16, name=f"pb{it}_{j}",
                                    tag="pb", bufs=7)
                    psl = (slice(0, 1024) if halves == (0, 1)
                           else slice(halves[0] * 512, halves[0] * 512 + 512))
                    nc.scalar.activation(pb[:, psl], pS[:, psl], Exp,
                                         bias=0.0, scale=1.0)
                    if not fold:
                        c0 = i0 - j0 + (S - 128)
                        w0, w1 = psl.start, psl.stop
                        if hh == 0:
                            eb = btab0[:, c0 - BT0_OFF + w0:c0 - BT0_OFF + w1]
                        else:
                            eb = btab1[:, c0 + w0:c0 + w1]
                        with nc.allow_low_precision(reason="fp16 probs"):
                            nc.vector.tensor_tensor(pb[:, psl], pb[:, psl],
                                                    eb, MUL)
                    if len(pend) == 5:
                        flush_pv()
                    pend.append((pb, j, halves))
                    if side:
                        side.pop(0)()
                while pend:
                    flush_pv()
                # normalization: oT = pacc[0:64] * (1/rowsum). pacc is freed
                # by the two copies below; the reciprocal round-trip and the
                # in-place multiply run off the critical path (reciprocal in
                # [8,128] layout, row<->col reshapes on DMA, broadcast on the
                # idle GpSimd engine).
                sumr = wk.tile([1, 1024], F32, name=f"sr{it}", tag="sumr",
                               bufs=1)
                nc.vector.tensor_copy(sumr[:], pacc[64:65, :])
                osl = oT[hb:hb + 64, b * 2048 + i0: b * 2048 + i0 + 1024]
                with nc.allow_low_precision(reason="bf16 out"):
                    nc.vector.tensor_copy(osl, pacc[0:64, :])
                sumc = wk.tile([8, 128], F32, name=f"sc{it}", tag="sumc")
                nc.sync.dma_start(sumc[:],
                                  sumr[:].rearrange("o (p a) -> o p a", a=128))
                inv8 = wk.tile([8, 128], F32, name=f"i8{it}", tag="inv8")
                nc.vector.reciprocal(inv8[:], sumc[:])
                invr = wk.tile([1, 1024], F32, name=f"iv{it}", tag="invr",
                               bufs=1)
                nc.sync.dma_start(invr[:].rearrange("o (p a) -> o p a", a=128),
                                  inv8[:])
                invbc = wk.tile([128, 1024], F32, name=f"ib{it}", tag="invbc",
                                bufs=1)
                nc.gpsimd.partition_broadcast(invbc[:], invr[:], channels=128)
                with nc.allow_low_precision(reason="bf16 out"):
                    nc.vector.tensor_tensor(osl, osl, invbc[hb:hb + 64, :],
                                            MUL)
                while side:
                    side.pop(0)()

            def out_proj_chunks(b, ih):
                def one(tloc):
                    def go():
                        tb = b * 16 + ih * 8 + tloc
                        ytile = wk.tile([128, 1024], F16, name=f"yt{tb}",
                                        tag="ytile", bufs=3)
                        for cq in range(2):
                            py_ = ps.tile([128, 512], F32,
                                          name=f"py{tb}_{cq}", tag="py")
                            nc.tensor.matmul(
                                py_[:],
                                oT[:, tb * 128:(tb + 1) * 128],
                                wo_sb[:, cq * 512:(cq + 1) * 512],
                                start=True, stop=True)
                            # split the evacuation: DVE is the binding
                            # engine in the out_proj windows, Scalar has
                            # slack there
                            if cq == 0:
                                with nc.allow_low_precision(reason="fp16 y"):
                                    nc.vector.tensor_copy(
                                        ytile[:, 0:512], py_[:])
                            else:
                                nc.scalar.activation(
                                    ytile[:, 512:1024], py_[:], Ident,
                                    bias=0.0, scale=1.0)
                            nc.sync.dma_start(
                                y_r[tb][:, cq * 512:(cq + 1) * 512],
                                ytile[:, cq * 512:(cq + 1) * 512])
                    return go
                return [one(tloc) for tloc in range(8)]

            ip0 = in_proj_chunks(0, evac_dve=False)
            ip0[0]()  # x tile 0 DMA — 2nd/3rd descriptors in line
            nc.sync.dma_start(wq_sb[:, 1:8, :], wqkvt_r[:, 1:8, :])
            nc.sync.dma_start(bq_sb[:], bqkv)
            for c in ip0[1:]:
                c()
            load_tables()
            for c in vt_chunks(0, tag_alt=True):
                c()
            ip1 = in_proj_chunks(1, evac_dve=True)
            attn_iter(0, 0, 0, side=ip1[:8])
            attn_iter(0, 0, 1)
            attn_iter(0, 1, 0, side=ip1[8:])
            vt1 = vt_chunks(1, tag_alt=False)
            attn_iter(0, 1, 1, side=vt1)
            attn_iter(1, 0, 0)
            attn_iter(1, 0, 1, side=out_proj_chunks(0, 0))
            attn_iter(1, 1, 0, side=out_proj_chunks(0, 1))
            attn_iter(1, 1, 1, side=out_proj_chunks(1, 0))
            for c in out_proj_chunks(1, 1):
                c()

    nc.compile()
    return nc


def _make_inmaps(x, in_proj_weight, in_proj_bias, out_proj_weight):
    import ml_dtypes
    bf16 = ml_dtypes.bfloat16

    slopes = _slopes()
    xT = np.ascontiguousarray(
        x.reshape(TOK, C).T.astype(bf16))  # [C, TOK]

    in_maps = []
    p = np.arange(128, dtype=np.float64)[:, None]
    cc = np.arange(BTW, dtype=np.float64)[None, :]
    for c in range(NCORE):
        heads = (c, c + 8)
        rows = []
        for sec in range(3):  # q, k, v
            for h in heads:
                rows.extend(range(sec * C + h * D, sec * C + (h + 1) * D))
        rows = np.array(rows)
        wq = in_proj_weight[rows, :].astype(np.float32).copy()
        bq = in_proj_bias[rows].astype(np.float32).copy()
        wq[:128] *= SCALE  # fold q scaling
        bq[:128] *= SCALE
        wqkvt = np.ascontiguousarray(wq.T.astype(bf16))  # [C, 384]
        bqkv = np.ascontiguousarray(bq.reshape(3, 128).T)  # [128, 3]

        btarr = np.empty((2, 128, BTW), dtype=np.float16)
        for hh, h in enumerate(heads):
            bias = np.minimum(float(slopes[h]) * (cc - (S - 128) - p),
                              float(MAX_BIAS))
            btarr[hh] = np.exp(bias - float(MAX_BIAS)).astype(np.float16)

        ocols = np.array(
            [heads[0] * D + d for d in range(D)]
            + [heads[1] * D + d for d in range(D)]
        )
        wotr = np.ascontiguousarray(
            out_proj_weight[:, ocols].T.astype(bf16))  # [128, C]

        in_maps.append({
            "xt": xT,
            "wqkvt": wqkvt,
            "bqkv": bqkv,
            "bt": btarr,
            "wot": wotr,
        })
    return in_maps


def run(inputs: dict, trace: bool = False):
    from concourse.bass_utils import run_bass_kernel_spmd

    nc = _program()
    in_maps = _make_inmaps(
        np.asarray(inputs["x"]),
        np.asarray(inputs["in_proj_weight"]),
        np.asarray(inputs["in_proj_bias"]),
        np.asarray(inputs["out_proj_weight"]),
    )
    res = run_bass_kernel_spmd(nc, in_maps, list(range(NCORE)), trace=trace)
    acc = np.zeros((TOK, C), dtype=np.float64)
    for r in res.results:
        acc += r["y"].astype(np.float64)
    acc += np.asarray(inputs["out_proj_bias"]).astype(np.float64)[None, :]
    out = acc.astype(np.float32).reshape(B, S, C)
    return out, res


def kernel(**inputs) -> np.ndarray:
    return run(inputs, trace=False)[0]
